# revision 7
# baseline (speedup 1.0000x reference)
"""Trainium2 Bass kernel for single-head attention (B=8, S=2048, E=768).

Data-parallel over batch: core c computes batch c entirely.

Host-side packing:
  Wkq  = Wk.T @ Wq          (fp8e4)  -- q/k projections fused into scores
  wvoa = (Wo @ Wv).T padded to [E, 769] (bf16); col 768 is zero
  boa  = [bo, 1]            (f32)    -- the 1 at col 768 builds the
                                        softmax-denominator "ones column"
  queryT/keyT quantized to fp8e4 and transposed to [E, *]; key/value
  gathered to the unmasked set (padded with masked keys -> exp(-200)=0).
  value transposed to valueT [E, nkeys] bf16.

Device dataflow (PE contraction dim = partition dim):
  Vp[j,o]  = sum_e valueT[e,j] wvoa[e,o] + boa[o]   (bf16; col 768 == 1)
  Hk[e',j] = sum_e Wkq[e,e'] keyT[e,j]              (fp8 DoubleRow, cast fp8)
  sT[j,i]  = sum_e' Hk[e',j] queryT[e',i]           (fp8 DoubleRow)
  aT[j,i]  = exp(sT/768 + maskbias[j])              (ACT, bf16)
  U[i,o]   = sum_j aT[j,i] Vp[j,o]                  (aT tiles as weights ->
                                                     output in [i,o] layout;
                                                     U[i,768] = den[i])
  y[i,:]   = U[i,:768] * (1/U[i,768])               (DVE mult w/ bcast recip)
"""

import numpy as np

S, E, P = 2048, 768, 128
NE, NS = E // P, S // P    # 6, 16
IC = 512                   # attention i-chunk
NIC = S // IC              # 4
N_CORES = 8
NKC = 1152                 # compacted key count (9 j-tiles); P(>NKC) ~ 1e-8
OA = 769                   # output width: 768 outputs + den col
OCH = ((0, 512), (512, OA - 512))

_CACHE = {}


def _chunks(total, step=512):
    out = []
    o = 0
    while o < total:
        out.append((o, min(step, total - o)))
        o += step
    return out


def build_nc(n_iters=1, nkeys=NKC):
    from contextlib import ExitStack

    import concourse.bacc as bacc
    import concourse.bass as bass
    import concourse.mybir as mybir
    import concourse.tile as tile

    F32 = mybir.dt.float32
    F32R = mybir.dt.float32r
    BF16 = mybir.dt.bfloat16
    F8 = mybir.dt.float8e4
    I32 = mybir.dt.int32
    AF = mybir.ActivationFunctionType
    ALU = mybir.AluOpType
    DR = mybir.MatmulPerfMode.DoubleRow

    KJ = nkeys // P
    NEP = NE // 2
    nc = bacc.Bacc("TRN2", target_bir_lowering=False, debug=False,
                   num_devices=N_CORES)

    xq_d = nc.dram_tensor("queryT8", [E, S], F8, kind="ExternalInput").ap()
    xk_d = nc.dram_tensor("keyT8", [E, nkeys], F8, kind="ExternalInput").ap()
    wkq_d = nc.dram_tensor("wkq8", [E, E], F8, kind="ExternalInput").ap()
    vt_d = nc.dram_tensor("valueT", [E, nkeys], BF16,
                          kind="ExternalInput").ap()
    wvo_d = nc.dram_tensor("wvoa", [E, OA], BF16, kind="ExternalInput").ap()
    boa_d = nc.dram_tensor("boa", [OA], F32, kind="ExternalInput").ap()
    mask_d = nc.dram_tensor("mask", [nkeys], I32, kind="ExternalInput").ap()
    y_d = nc.dram_tensor("out", [S, E], F32, kind="ExternalOutput").ap()

    # double-buffer all SBUF pools across iterations (the n_iters>1 variants
    # exist for marginal-cost timing): iteration N+1's input DMAs then write
    # different addresses than anything iteration N still reads, so prefetch
    # overlaps N's attention. The nkeys==S fallback doesn't fit SBUF at
    # bufs=2; it runs single-buffered (rare, perf-irrelevant).
    DB = 2 if (nkeys == NKC and n_iters > 1) else 1
    with tile.TileContext(nc) as tc, \
         tc.tile_pool(name="persist", bufs=DB) as persist, \
         tc.tile_pool(name="wt", bufs=DB) as wt_pool, \
         tc.tile_pool(name="at", bufs=2) as at_pool, \
         tc.tile_pool(name="rc", bufs=4) as rc_pool, \
         tc.tile_pool(name="ys", bufs=3) as y_pool, \
         tc.tile_pool(name="ps_s", bufs=4, space="PSUM") as ps_s, \
         tc.tile_pool(name="ps_u", bufs=2, space="PSUM") as ps_u:
      for _it in range(n_iters):
        xq8 = persist.tile([P, NE, S], F8, tag="xq")
        hk8 = persist.tile([P, NE, nkeys], F8, tag="hk")
        vp = persist.tile([P, KJ, OA], BF16, tag="vp")
        maskb = persist.tile([P, KJ], F32, tag="mb")
        boa_rep = persist.tile([P, OA], F32, tag="boa")

        # ---------- phase A: Hk (fp8 DoubleRow) + Vp = vT.T @ wvoa ----------
        if True:
            # DMA order drives the early pipeline: Hk inputs first, then
            # Vp inputs (j-chunked so Vp streams), query last (needed ~30us).
            wkq8 = wt_pool.tile([P, NE, E], F8, tag="wkq")
            xk8 = wt_pool.tile([P, NE, nkeys], F8, tag="xk")
            vt_sb = wt_pool.tile([P, NE, nkeys], BF16, tag="vt")
            wvo_sb = wt_pool.tile([P, NE, OA], BF16, tag="wvo")

            for t in range(NEP):       # Hk inputs first, e-pair chunked:
                nc.sync.dma_start(     # first Hk matmul starts after ~0.6MB
                    out=wkq8[:, 2 * t:2 * t + 2, :],
                    in_=wkq_d[2 * t * P:(2 * t + 2) * P, :].rearrange(
                        "(t p) o -> p t o", p=P))
                nc.sync.dma_start(
                    out=xk8[:, 2 * t:2 * t + 2, :],
                    in_=xk_d[2 * t * P:(2 * t + 2) * P, :].rearrange(
                        "(t p) j -> p t j", p=P))
            # first query chunk next: scores(ic0) fills the PE while the
            # (larger) Vp inputs stream in; mask/bias ride along (tiny, and
            # exp(ic0) needs maskb before the Vp inputs finish)
            nc.sync.dma_start(out=xq8[:, :, 0:IC],
                              in_=xq_d[:, 0:IC].rearrange(
                                  "(t p) i -> p t i", p=P))
            def mask_boa_dmas():
                boa_bc = bass.AP(tensor=boa_d.tensor, offset=boa_d.offset,
                                 ap=[[0, P]] + list(boa_d.ap))
                nc.sync.dma_start(out=boa_rep, in_=boa_bc)
                nc.sync.dma_start(out=mask_sb,
                                  in_=mask_d.rearrange("(t p) -> p t", p=P))

            mask_sb = persist.tile([P, KJ], I32, tag="msk")
            if _it == 0:
                # cold start: exp(ic0) needs maskb before the Vp inputs
                # finish streaming, so these tiny DMAs go early
                mask_boa_dmas()
            nc.sync.dma_start(
                out=wvo_sb, in_=wvo_d.rearrange("(t p) o -> p t o", p=P))
            for j0, jn in _chunks(nkeys, 3 * P):
                nc.sync.dma_start(
                    out=vt_sb[:, :, j0:j0 + jn],
                    in_=vt_d[:, j0:j0 + jn].rearrange(
                        "(t p) j -> p t j", p=P))
            nc.sync.dma_start(out=xq8[:, :, IC:],
                              in_=xq_d[:, IC:].rearrange(
                                  "(t p) i -> p t i", p=P))
            if _it > 0:
                # steady state: iteration N+1's mask prefetches during N's
                # attention anyway; keep the big transfers at queue head
                mask_boa_dmas()

            mask_f = persist.tile([P, KJ], F32, tag="mskf")
            nc.vector.tensor_copy(out=mask_f, in_=mask_sb)
            nc.vector.tensor_scalar(out=maskb, in0=mask_f, scalar1=200.0,
                                    scalar2=-200.0, op0=ALU.mult,
                                    op1=ALU.add)

            for ept in range(NE):      # e' tile of Hk rows
                # t-outer, chunks inner: each DoubleRow weight pair loads
                # once and serves all three j-chunks (DR ldweights are not
                # FWL-hidden on hardware, so fewer loads matter there)
                chunks = _chunks(nkeys)
                hps = [ps_s.tile([P, 512], F32, tag="s",
                                 name=f"hk{_it}_{ept}_{o0}")
                       for o0, _ in chunks]
                for t in range(NEP):
                    for hp, (o0, on) in zip(hps, chunks):
                        nc.tensor.matmul(
                            hp[:, :on],
                            lhsT=wkq8[:, 2 * t:2 * t + 2,
                                      ept * P:(ept + 1) * P],
                            rhs=xk8[:, 2 * t:2 * t + 2, o0:o0 + on],
                            perf_mode=DR,
                            start=(t == 0), stop=(t == NEP - 1))
                for hp, (o0, on) in zip(hps, chunks):
                    # casts on ACT: DVE stays free for the Vp adds
                    nc.scalar.copy(hk8[:, ept, o0:o0 + on], hp[:, :on])

            at_tiles = {}

            def scores_block(ic):
                isl = slice(ic * IC, (ic + 1) * IC)
                at_all = at_pool.tile([P, KJ, IC], BF16, tag="at")
                at_tiles[ic] = at_all
                for jt in range(KJ):
                    sp = ps_s.tile([P, 512], F32, tag="s",
                                   name=f"sp{_it}_{ic}_{jt}")
                    for t in range(NEP):
                        nc.tensor.matmul(
                            sp,
                            lhsT=hk8[:, 2 * t:2 * t + 2,
                                     jt * P:(jt + 1) * P],
                            rhs=xq8[:, 2 * t:2 * t + 2, isl],
                            perf_mode=DR,
                            start=(t == 0), stop=(t == NEP - 1))
                    nc.scalar.activation(
                        out=at_all[:, jt, :], in_=sp, func=AF.Exp,
                        bias=maskb[:, jt:jt + 1], scale=1.0 / float(E))

            def u_block(ic):
                at_all = at_tiles.pop(ic)
                for it in range(IC // P):
                    up = ps_u.tile([P, OA], F32, tag="u",
                                   name=f"u{_it}_{ic}_{it}")
                    for jt in range(KJ):
                        for q0, qn in OCH:
                            nc.tensor.matmul(
                                up[:, q0:q0 + qn],
                                lhsT=at_all[:, jt, it * P:(it + 1) * P],
                                rhs=vp[:, jt, q0:q0 + qn],
                                start=(jt == 0), stop=(jt == KJ - 1))
                    recip = rc_pool.tile([P, 1], F32, tag="rc")
                    nc.vector.reciprocal(recip, up[:, E:E + 1])
                    ysb = y_pool.tile([P, E], F32, tag="y")
                    r0 = ic * IC + it * P
                    # out-DMAs issue from gpsimd: SP stays free so the next
                    # iteration's input DMAs prefetch during attention. The
                    # program's final tile is split in half and issued on SP
                    # (hwdge latency < swdge; first half's DMA overlaps the
                    # second half's normalize) to shorten the drain tail.
                    last = (_it == n_iters - 1 and ic == NIC - 1
                            and it == IC // P - 1)
                    halves = ((0, E // 2), (E // 2, E // 2)) if last \
                        else ((0, E),)
                    eng = nc.sync if last else nc.gpsimd
                    for o0, on in halves:
                        recip_bc = bass.AP(tensor=recip.tensor,
                                           offset=recip.offset,
                                           ap=[recip.ap[0], [0, on]])
                        nc.vector.tensor_tensor(
                            out=ysb[:, o0:o0 + on], in0=up[:, o0:o0 + on],
                            in1=recip_bc, op=ALU.mult)
                        eng.dma_start(out=y_d[r0:r0 + P, o0:o0 + on],
                                      in_=ysb[:, o0:o0 + on])

            # scores(ic0) sits between Hk and Vp: it only needs hk8 and the
            # first query chunk, and fills the PE while Vp's inputs stream
            scores_block(0)

            for jt in range(KJ):
                up = ps_u.tile([P, OA], F32, tag="u", name=f"vp{_it}_{jt}")
                for et in range(NE):
                    for q0, qn in OCH:
                        nc.tensor.matmul(
                            up[:, q0:q0 + qn],
                            lhsT=vt_sb[:, et, jt * P:(jt + 1) * P],
                            rhs=wvo_sb[:, et, q0:q0 + qn],
                            start=(et == 0), stop=(et == NE - 1))
                nc.vector.tensor_tensor(out=vp[:, jt, :], in0=up,
                                        in1=boa_rep, op=ALU.add)

        # ---------------- phase B: attention + output ----------------
        if True:
            for ic in range(NIC):
                u_block(ic)
                if ic + 1 < NIC:
                    scores_block(ic + 1)

    nc.compile()
    return nc


def get_nc(n_iters=1, nkeys=NKC):
    key = ("nc", n_iters, nkeys)
    if key not in _CACHE:
        _CACHE[key] = build_nc(n_iters, nkeys)
    return _CACHE[key]


def pack_inputs(value, key, query, mask, Wv, Wk, Wq, Wo, bo):
    """Host-side packing: per-core input maps (weight fusion + layouts)."""
    import ml_dtypes

    F8 = ml_dtypes.float8_e4m3

    value = np.asarray(value, dtype=np.float32)
    key = np.asarray(key, dtype=np.float32)
    query = np.asarray(query, dtype=np.float32)
    mask = np.asarray(mask, dtype=np.int32)
    Wv = np.asarray(Wv, dtype=np.float32)
    Wk = np.asarray(Wk, dtype=np.float32)
    Wq = np.asarray(Wq, dtype=np.float32)
    Wo = np.asarray(Wo, dtype=np.float32)
    bo = np.asarray(bo, dtype=np.float32)

    wkq8 = np.ascontiguousarray(Wk.T @ Wq).astype(F8)
    wvoa = np.zeros((E, OA), dtype=ml_dtypes.bfloat16)
    wvoa[:, :E] = ((Wo @ Wv).T).astype(ml_dtypes.bfloat16)
    boa = np.zeros(OA, dtype=np.float32)
    boa[:E] = bo
    boa[E] = 1.0

    # key compaction: keep unmasked keys, pad with masked ones (exp -> 0)
    idxs = []
    nkeys = NKC
    for c in range(N_CORES):
        m = mask[c, 0]
        keep = np.flatnonzero(m != 0)
        drop = np.flatnonzero(m == 0)
        if len(keep) > NKC or len(drop) == 0:
            nkeys = S
            break
        pad = np.full(NKC - len(keep), drop[0], dtype=np.int64)
        idxs.append(np.concatenate([keep, pad]))

    in_maps = []
    for c in range(N_CORES):
        if nkeys == S:
            kc, vc, mc = key[c], value[c], mask[c, 0]
        else:
            ix = idxs[c]
            kc, vc, mc = key[c][ix], value[c][ix], mask[c, 0][ix]
        in_maps.append({
            "queryT8": np.ascontiguousarray(query[c].T).astype(F8),
            "keyT8": np.ascontiguousarray(kc.T).astype(F8),
            "wkq8": wkq8,
            "valueT": np.ascontiguousarray(vc.T).astype(ml_dtypes.bfloat16),
            "wvoa": wvoa,
            "boa": boa,
            "mask": np.ascontiguousarray(mc),
        })
    return in_maps, nkeys


def kernel(**inputs):
    from concourse.bass_utils import run_bass_kernel_spmd

    in_maps, nkeys = pack_inputs(
        inputs["value"], inputs["key"], inputs["query"], inputs["mask"],
        inputs["Wv"], inputs["Wk"], inputs["Wq"], inputs["Wo"], inputs["bo"])
    nc = get_nc(nkeys=nkeys)
    res = run_bass_kernel_spmd(nc, in_maps, list(range(N_CORES)))
    out = np.stack([res.results[c]["out"] for c in range(N_CORES)], axis=0)
    return out
